# revision 10
# baseline (speedup 1.0000x reference)
"""Trainium2 Bass kernel for quantized cosine-distance (1 - cos similarity).

Math: the reference bit-slices 7-bit symmetric-quantized, L2-normalized inputs
into (1,2,4)-bit groups and recombines 9 low-bit GEMMs with power-of-two
weights.  That recombination is exactly  qx @ qw^T  with qx, qw integer
matrices in [-127, 127].  Those integers are exact in bf16 and every partial
dot product over D=1024 is < 2^24, so a single bf16 matmul with f32 PSUM
accumulation reproduces the 9-GEMM result exactly.

Kernel structure (8 NeuronCores, weight sharded along M, x replicated):
  Launch 1 (tiny): per-core row stats (1/norm, max|row|/norm) for its x slice
      and weight shard.  Host only gathers shards and takes max of 8 scalars.
  Launch 2 (main): quantize x and w-shard in transposed layout, one big bf16
      GEMM per core -> [B, M/8] block, epilogue 1 - s*acc, DMA out.
"""

import os

import numpy as np

import concourse.bass as bass
import concourse.mybir as mybir
import concourse.tile as tile
from concourse import bacc
from concourse.bass_isa import ReduceOp
from concourse.bass_utils import run_bass_kernel_spmd

F32 = mybir.dt.float32
BF16 = mybir.dt.bfloat16
AF = mybir.ActivationFunctionType
ALU = mybir.AluOpType
AX = mybir.AxisListType

N_CORES = 8
B_FULL = 4096
D_FULL = 1024
M_FULL = 8192
P = 128

# magic constant: adding then subtracting 1.5*2^23 rounds |v|<2^22 to the
# nearest integer (ties-to-even), matching jnp.round for our value range
KMAG = float(np.float32(1.5 * 2**23))

# set by test.py to capture a profile of the main launch (NTFF hook is not
# available in all containers; falls back to no trace)
TRACE = bool(int(os.environ.get("COSDIST_TRACE", "0")))
LAST = {}
_PROGRAM_CACHE = {}


def _cached_program(key, builder):
    if key not in _PROGRAM_CACHE:
        _PROGRAM_CACHE[key] = builder()
    return _PROGRAM_CACHE[key]


def _f32(a):
    return np.ascontiguousarray(np.asarray(a, dtype=np.float32))


# --------------------------------------------------------------------------
# Launch 1: row stats.  Inputs per core: x_sl [B_SL, D], w_sh [M_SH, D].
# Outputs: rnorm (1/max(||row||,1e-12)) in [P, ntiles] partition-major layout
# and the per-core max of (max|row| / ||row||) as [1, 1].
# --------------------------------------------------------------------------
def build_stats_program(b_sl, m_sh, d):
    nc = bacc.Bacc("TRN2", target_bir_lowering=False, debug=False)
    x_sl = nc.dram_tensor("x_sl", [b_sl, d], F32, kind="ExternalInput")
    w_sh = nc.dram_tensor("w_sh", [m_sh, d], F32, kind="ExternalInput")
    x_rn = nc.dram_tensor("x_rn", [P, b_sl // P], F32, kind="ExternalOutput")
    x_rm = nc.dram_tensor("x_rm", [1, 1], F32, kind="ExternalOutput")
    w_rn = nc.dram_tensor("w_rn", [P, m_sh // P], F32, kind="ExternalOutput")
    w_rm = nc.dram_tensor("w_rm", [1, 1], F32, kind="ExternalOutput")

    with tile.TileContext(nc) as tc:
        with (
            tc.tile_pool(name="work", bufs=3) as work,
            tc.tile_pool(name="stat", bufs=1) as stat,
        ):
            for inp, nt, rn_out, rm_out, pre in (
                (x_sl, b_sl // P, x_rn, x_rm, "x"),
                (w_sh, m_sh // P, w_rn, w_rm, "w"),
            ):
                ssq = stat.tile([P, nt], F32, tag=f"{pre}ssq")
                amax = stat.tile([P, nt], F32, tag=f"{pre}amax")
                for t in range(nt):
                    xt = work.tile([P, d], F32, tag="xt")
                    nc.sync.dma_start(xt[:], inp[t * P : (t + 1) * P, :])
                    sq = work.tile([P, d], F32, tag="sq")
                    nc.vector.tensor_mul(sq[:], xt[:], xt[:])
                    nc.vector.tensor_reduce(
                        ssq[:, t : t + 1], sq[:], axis=AX.X, op=ALU.add
                    )
                    nc.vector.tensor_reduce(
                        amax[:, t : t + 1],
                        xt[:],
                        axis=AX.X,
                        op=ALU.max,
                        apply_absolute_value=True,
                    )
                norm = stat.tile([P, nt], F32, tag=f"{pre}norm")
                nc.scalar.sqrt(norm[:], ssq[:])
                nc.vector.tensor_scalar_max(norm[:], norm[:], 1e-12)
                rnorm = stat.tile([P, nt], F32, tag=f"{pre}rn")
                nc.vector.reciprocal(rnorm[:], norm[:])
                ratio = stat.tile([P, nt], F32, tag=f"{pre}ratio")
                nc.vector.tensor_mul(ratio[:], amax[:], rnorm[:])
                rmax = stat.tile([P, 1], F32, tag=f"{pre}rmax")
                nc.vector.tensor_reduce(rmax[:], ratio[:], axis=AX.X, op=ALU.max)
                gmax = stat.tile([P, 1], F32, tag=f"{pre}gmax")
                nc.gpsimd.partition_all_reduce(gmax[:], rmax[:], P, ReduceOp.max)
                nc.sync.dma_start(rn_out[:], rnorm[:])
                nc.sync.dma_start(rm_out[:], gmax[0:1, 0:1])
    nc.compile()
    return nc


# --------------------------------------------------------------------------
# Launch 2: quantize + GEMM + epilogue.
# Inputs per core (all transposed layouts prepared host-side):
#   xT   [D, B]     x transposed (replicated)
#   wT   [D, M_SH]  weight shard transposed
#   rnx  [1, B]     1/norm per x row (full)
#   rnw  [1, M_SH]  1/norm per weight row (this shard)
#   sx   [1, 1]     global max|xn|;  sw [1, 1] likewise for w
# Output: out [B, M_SH] = 1 - (sx/127)*(sw/127) * (qx @ qw^T) block
# --------------------------------------------------------------------------
def build_main_program(b, m_sh, d, n_free=512, b_chunk=512):
    nc = bacc.Bacc("TRN2", target_bir_lowering=False, debug=False)
    xT = nc.dram_tensor("xT", [d, b], F32, kind="ExternalInput")
    wT = nc.dram_tensor("wT", [d, m_sh], F32, kind="ExternalInput")
    rnx = nc.dram_tensor("rnx", [1, b], F32, kind="ExternalInput")
    rnw = nc.dram_tensor("rnw", [1, m_sh], F32, kind="ExternalInput")
    sx = nc.dram_tensor("sx", [1, 1], F32, kind="ExternalInput")
    sw = nc.dram_tensor("sw", [1, 1], F32, kind="ExternalInput")
    out = nc.dram_tensor("out", [b, m_sh], F32, kind="ExternalOutput")

    kb = d // P  # number of 128-deep contraction blocks
    nch = b // b_chunk  # b-chunks for pipelined x quantization
    nbt_per_ch = b_chunk // P  # 128-row output tiles per chunk
    nmt = m_sh // n_free  # output column tiles

    with tile.TileContext(nc) as tc:
        with (
            tc.tile_pool(name="dram", bufs=1, space="DRAM") as dram,
            tc.tile_pool(name="const", bufs=1) as cpool,
            tc.tile_pool(name="qx", bufs=1) as qxp,
            tc.tile_pool(name="qw", bufs=1) as qwp,
            tc.tile_pool(name="cx", bufs=3) as cxp,
            tc.tile_pool(name="xs", bufs=8) as xsp,
            tc.tile_pool(name="ws", bufs=2) as wsp,
            tc.tile_pool(name="scr", bufs=4) as scrp,
            tc.tile_pool(name="outp", bufs=6) as outp,
            tc.tile_pool(name="psum", bufs=6, space="PSUM") as psp,
        ):
            # ---- scale rows ----
            rnx_sb = cpool.tile([1, b], F32)
            rnw_sb = cpool.tile([1, m_sh], F32)
            sx_sb = cpool.tile([1, 1], F32)
            sw_sb = cpool.tile([1, 1], F32)
            nc.sync.dma_start(rnx_sb[:], rnx[:])
            nc.sync.dma_start(rnw_sb[:], rnw[:])
            nc.sync.dma_start(sx_sb[:], sx[:])
            nc.sync.dma_start(sw_sb[:], sw[:])

            # c = (rnorm / s) * 127   (quantization multiplier per row);
            # tensor_scalar has no divide op, so use reciprocal + mult
            rsx = cpool.tile([1, 1], F32)
            nc.vector.reciprocal(rsx[:], sx_sb[:])
            rsw = cpool.tile([1, 1], F32)
            nc.vector.reciprocal(rsw[:], sw_sb[:])
            nc.vector.tensor_scalar(
                rnx_sb[:], rnx_sb[:],
                scalar1=rsx[0:1, 0:1], scalar2=127.0,
                op0=ALU.mult, op1=ALU.mult,
            )
            nc.vector.tensor_scalar(
                rnw_sb[:], rnw_sb[:],
                scalar1=rsw[0:1, 0:1], scalar2=127.0,
                op0=ALU.mult, op1=ALU.mult,
            )
            # bounce via DRAM so the rows can be partition-broadcast by DMA
            cx_dram = dram.tile([1, b], F32)
            cw_dram = dram.tile([1, m_sh], F32)
            nc.sync.dma_start(cx_dram[:], rnx_sb[:])
            nc.sync.dma_start(cw_dram[:], rnw_sb[:])

            # epilogue scale: -(sx/127)*(sw/127), broadcast to all partitions
            nsxsw = cpool.tile([1, 1], F32)
            nc.vector.tensor_scalar(
                nsxsw[:], sx_sb[:],
                scalar1=sw_sb[0:1, 0:1], scalar2=-1.0 / (127.0 * 127.0),
                op0=ALU.mult, op1=ALU.mult,
            )
            nsxsw_b = cpool.tile([P, 1], F32)
            nc.gpsimd.partition_broadcast(nsxsw_b[:], nsxsw[:])

            # bias constants for the round-to-nearest magic trick
            kpos = cpool.tile([P, 1], F32)
            nc.vector.memset(kpos[:], KMAG)
            kneg = cpool.tile([P, 1], F32)
            nc.vector.memset(kneg[:], -KMAG)

            # ---- quantize weight shard: qwT[k] [P, m_sh] bf16 ----
            cw_full = cpool.tile([P, m_sh], F32)
            nc.sync.dma_start(cw_full[:], cw_dram[0:1, :].to_broadcast((P, m_sh)))
            qw_tiles = []
            for k in range(kb):
                wt = wsp.tile([P, m_sh], F32, tag="wt")
                nc.sync.dma_start(wt[:], wT[k * P : (k + 1) * P, :])
                tq = wsp.tile([P, m_sh], F32, tag="wtq")
                nc.vector.tensor_mul(tq[:], wt[:], cw_full[:])
                uq = wsp.tile([P, m_sh], F32, tag="wuq")
                nc.scalar.activation(uq[:], tq[:], AF.Identity, bias=kpos[:])
                qw_k = qwp.tile([P, m_sh], BF16, tag=f"qw{k}")
                nc.scalar.activation(qw_k[:], uq[:], AF.Identity, bias=kneg[:])
                qw_tiles.append(qw_k)

            # ---- quantize x chunk-by-chunk; matmul pipelined behind it ----
            qx_tiles = {}

            def quant_chunk(ch):
                cxf = cxp.tile([P, b_chunk], F32, tag="cxf")
                nc.sync.dma_start(
                    cxf[:],
                    cx_dram[0:1, ch * b_chunk : (ch + 1) * b_chunk].to_broadcast(
                        (P, b_chunk)
                    ),
                )
                for k in range(kb):
                    xt = xsp.tile([P, b_chunk], F32, tag="xt")
                    nc.sync.dma_start(
                        xt[:], xT[k * P : (k + 1) * P, ch * b_chunk : (ch + 1) * b_chunk]
                    )
                    tq = scrp.tile([P, b_chunk], F32, tag="xtq")
                    nc.vector.tensor_mul(tq[:], xt[:], cxf[:])
                    uq = scrp.tile([P, b_chunk], F32, tag="xuq")
                    nc.scalar.activation(uq[:], tq[:], AF.Identity, bias=kpos[:])
                    qx_k = qxp.tile([P, b_chunk], BF16, tag=f"qx{k}_{ch}")
                    nc.scalar.activation(qx_k[:], uq[:], AF.Identity, bias=kneg[:])
                    qx_tiles[(k, ch)] = qx_k

            quant_chunk(0)
            if nch > 1:
                quant_chunk(1)
            for ch in range(nch):
                for bt in range(nbt_per_ch):
                    pss = [
                        psp.tile([P, n_free], F32, tag="mm", name=f"mm_{ch}_{bt}_{i}")
                        for i in range(nmt)
                    ]
                    lo = bt * P
                    for k in range(kb):
                        lhsT = qx_tiles[(k, ch)][:, lo : lo + P]
                        for mt in range(nmt):
                            nc.tensor.matmul(
                                pss[mt][:],
                                lhsT,
                                qw_tiles[k][:, mt * n_free : (mt + 1) * n_free],
                                start=(k == 0),
                                stop=(k == kb - 1),
                            )
                    row = ch * b_chunk + bt * P
                    for mt in range(nmt):
                        ot = outp.tile([P, n_free], F32, tag="ot")
                        nc.vector.tensor_scalar(
                            ot[:], pss[mt][:],
                            scalar1=nsxsw_b[:], scalar2=1.0,
                            op0=ALU.mult, op1=ALU.add,
                        )
                        nc.sync.dma_start(
                            out[row : row + P, mt * n_free : (mt + 1) * n_free], ot[:]
                        )
                if ch + 2 < nch:
                    quant_chunk(ch + 2)
    nc.compile()
    return nc


# --------------------------------------------------------------------------
# host orchestration
# --------------------------------------------------------------------------
def _pm_to_vec(a):
    """[P, nt] partition-major stats tile -> flat row vector (b = t*P + p)."""
    return np.ascontiguousarray(a.T).reshape(-1)


def kernel(x, weight):
    x = _f32(x)
    w = _f32(weight)
    b, d = x.shape
    m, d2 = w.shape
    assert (b, d, m, d2) == (B_FULL, D_FULL, M_FULL, D_FULL), (x.shape, w.shape)
    b_sl = b // N_CORES
    m_sh = m // N_CORES
    cores = list(range(N_CORES))

    # ---- launch 1: stats ----
    nc1 = _cached_program("stats", lambda: build_stats_program(b_sl, m_sh, d))
    in1 = [
        {
            "x_sl": np.ascontiguousarray(x[c * b_sl : (c + 1) * b_sl]),
            "w_sh": np.ascontiguousarray(w[c * m_sh : (c + 1) * m_sh]),
        }
        for c in cores
    ]
    res1 = run_bass_kernel_spmd(nc1, in1, core_ids=cores).results

    rn_x = np.concatenate([_pm_to_vec(res1[c]["x_rn"]) for c in cores])
    s_x = np.float32(max(np.float32(res1[c]["x_rm"][0, 0]) for c in cores))
    s_w = np.float32(max(np.float32(res1[c]["w_rm"][0, 0]) for c in cores))

    # ---- launch 2: quantize + matmul ----
    nc2 = _cached_program("main", lambda: build_main_program(b, m_sh, d))
    xT = np.ascontiguousarray(x.T)
    rnx_row = rn_x.reshape(1, b)
    sx_t = np.full((1, 1), s_x, dtype=np.float32)
    sw_t = np.full((1, 1), s_w, dtype=np.float32)
    in2 = []
    for c in cores:
        in2.append(
            {
                "xT": xT,
                "wT": np.ascontiguousarray(w[c * m_sh : (c + 1) * m_sh].T),
                "rnx": rnx_row,
                "rnw": _pm_to_vec(res1[c]["w_rn"]).reshape(1, m_sh),
                "sx": sx_t,
                "sw": sw_t,
            }
        )
    try:
        r = run_bass_kernel_spmd(nc2, in2, core_ids=cores, trace=TRACE)
    except ModuleNotFoundError:
        # axon NTFF profiling hook unavailable in this container
        r = run_bass_kernel_spmd(nc2, in2, core_ids=cores, trace=False)
    LAST["exec_time_ns"] = r.exec_time_ns
    LAST["mean_exec_time_ns"] = r.mean_exec_time_ns
    LAST["trace"] = r.instructions_and_trace[1] if r.instructions_and_trace else None
    LAST["in2"] = in2
    LAST["nc2"] = nc2

    return np.concatenate([r.results[c]["out"] for c in cores], axis=1)


# revision 13
# speedup vs baseline: 11.3110x; 11.3110x over previous
"""Trainium2 Bass kernel for quantized cosine-distance (1 - cos similarity).

Math: the reference bit-slices 7-bit symmetric-quantized, L2-normalized inputs
into (1,2,4)-bit groups and recombines 9 low-bit GEMMs with power-of-two
weights.  That recombination is exactly  qx @ qw^T  with qx, qw integer
matrices in [-127, 127].  Those integers are exact in bf16 and every partial
dot product over D=1024 is < 2^24, so a single bf16 matmul with f32 PSUM
accumulation reproduces the 9-GEMM result exactly.

Kernel structure (8 NeuronCores, weight sharded along M, x replicated):
  Launch 1 (tiny): per-core row stats (1/norm, max|row|/norm) for its x slice
      and weight shard.  Host only gathers shards and takes max of 8 scalars.
  Launch 2 (main): quantize x and w-shard in transposed layout, one big bf16
      GEMM per core -> [B, M/8] block, epilogue 1 - s*acc, DMA out.
"""

import os

import numpy as np

import concourse.bass as bass
import concourse.mybir as mybir
import concourse.tile as tile
from concourse import bacc
from concourse.bass_isa import ReduceOp
from concourse.bass_utils import run_bass_kernel_spmd

F32 = mybir.dt.float32
BF16 = mybir.dt.bfloat16
AF = mybir.ActivationFunctionType
ALU = mybir.AluOpType
AX = mybir.AxisListType

N_CORES = 8
B_FULL = 4096
D_FULL = 1024
M_FULL = 8192
P = 128

# magic constant: adding then subtracting 1.5*2^23 rounds |v|<2^22 to the
# nearest integer (ties-to-even), matching jnp.round for our value range
KMAG = float(np.float32(1.5 * 2**23))

# set by test.py to capture a profile of the main launch (NTFF hook is not
# available in all containers; falls back to no trace)
TRACE = bool(int(os.environ.get("COSDIST_TRACE", "0")))
LAST = {}
_PROGRAM_CACHE = {}


def _cached_program(key, builder):
    if key not in _PROGRAM_CACHE:
        _PROGRAM_CACHE[key] = builder()
    return _PROGRAM_CACHE[key]


def _f32(a):
    return np.ascontiguousarray(np.asarray(a, dtype=np.float32))


# --------------------------------------------------------------------------
# Launch 1: row stats.  Inputs per core: x_sl [B_SL, D], w_sh [M_SH, D].
# Outputs: rnorm (1/max(||row||,1e-12)) in [P, ntiles] partition-major layout
# and the per-core max of (max|row| / ||row||) as [1, 1].
# --------------------------------------------------------------------------
def build_stats_program(b_sl, m_sh, d):
    nc = bacc.Bacc("TRN2", target_bir_lowering=False, debug=False)
    x_sl = nc.dram_tensor("x_sl", [b_sl, d], F32, kind="ExternalInput")
    w_sh = nc.dram_tensor("w_sh", [m_sh, d], F32, kind="ExternalInput")
    x_rn = nc.dram_tensor("x_rn", [P, b_sl // P], F32, kind="ExternalOutput")
    x_rm = nc.dram_tensor("x_rm", [1, 1], F32, kind="ExternalOutput")
    w_rn = nc.dram_tensor("w_rn", [P, m_sh // P], F32, kind="ExternalOutput")
    w_rm = nc.dram_tensor("w_rm", [1, 1], F32, kind="ExternalOutput")

    with tile.TileContext(nc) as tc:
        with (
            tc.tile_pool(name="work", bufs=3) as work,
            tc.tile_pool(name="stat", bufs=1) as stat,
        ):
            for inp, nt, rn_out, rm_out, pre in (
                (x_sl, b_sl // P, x_rn, x_rm, "x"),
                (w_sh, m_sh // P, w_rn, w_rm, "w"),
            ):
                ssq = stat.tile([P, nt], F32, tag=f"{pre}ssq")
                amax = stat.tile([P, nt], F32, tag=f"{pre}amax")
                for t in range(nt):
                    xt = work.tile([P, d], F32, tag="xt")
                    nc.sync.dma_start(xt[:], inp[t * P : (t + 1) * P, :])
                    sq = work.tile([P, d], F32, tag="sq")
                    nc.vector.tensor_mul(sq[:], xt[:], xt[:])
                    nc.vector.tensor_reduce(
                        ssq[:, t : t + 1], sq[:], axis=AX.X, op=ALU.add
                    )
                    nc.vector.tensor_reduce(
                        amax[:, t : t + 1],
                        xt[:],
                        axis=AX.X,
                        op=ALU.max,
                        apply_absolute_value=True,
                    )
                norm = stat.tile([P, nt], F32, tag=f"{pre}norm")
                nc.scalar.sqrt(norm[:], ssq[:])
                nc.vector.tensor_scalar_max(norm[:], norm[:], 1e-12)
                rnorm = stat.tile([P, nt], F32, tag=f"{pre}rn")
                nc.vector.reciprocal(rnorm[:], norm[:])
                ratio = stat.tile([P, nt], F32, tag=f"{pre}ratio")
                nc.vector.tensor_mul(ratio[:], amax[:], rnorm[:])
                rmax = stat.tile([P, 1], F32, tag=f"{pre}rmax")
                nc.vector.tensor_reduce(rmax[:], ratio[:], axis=AX.X, op=ALU.max)
                gmax = stat.tile([P, 1], F32, tag=f"{pre}gmax")
                nc.gpsimd.partition_all_reduce(gmax[:], rmax[:], P, ReduceOp.max)
                nc.sync.dma_start(rn_out[:], rnorm[:])
                nc.sync.dma_start(rm_out[:], gmax[0:1, 0:1])
    nc.compile()
    return nc


# --------------------------------------------------------------------------
# Launch 2: quantize + GEMM + epilogue.
# Inputs per core (all transposed layouts prepared host-side):
#   xT   [D, B]     x transposed (replicated)
#   wT   [D, M_SH]  weight shard transposed
#   rnx  [1, B]     1/norm per x row (full)
#   rnw  [1, M_SH]  1/norm per weight row (this shard)
#   sx   [1, 1]     global max|xn|;  sw [1, 1] likewise for w
# Output: out [B, M_SH] = 1 - (sx/127)*(sw/127) * (qx @ qw^T) block
# --------------------------------------------------------------------------
def build_main_program(b, m_sh, d, n_free=512, b_chunk=512):
    nc = bacc.Bacc("TRN2", target_bir_lowering=False, debug=False)
    xT = nc.dram_tensor("xT", [d, b], F32, kind="ExternalInput")
    wT = nc.dram_tensor("wT", [d, m_sh], F32, kind="ExternalInput")
    rnx = nc.dram_tensor("rnx", [1, b], F32, kind="ExternalInput")
    rnw = nc.dram_tensor("rnw", [1, m_sh], F32, kind="ExternalInput")
    sx = nc.dram_tensor("sx", [1, 1], F32, kind="ExternalInput")
    sw = nc.dram_tensor("sw", [1, 1], F32, kind="ExternalInput")
    out = nc.dram_tensor("out", [b, m_sh], F32, kind="ExternalOutput")

    kb = d // P  # number of 128-deep contraction blocks
    nch = b // b_chunk  # b-chunks for pipelined x quantization
    nbt_per_ch = b_chunk // P  # 128-row output tiles per chunk
    nmt = m_sh // n_free  # output column tiles

    with tile.TileContext(nc) as tc:
        with (
            tc.tile_pool(name="dram", bufs=1, space="DRAM") as dram,
            tc.tile_pool(name="const", bufs=1) as cpool,
            tc.tile_pool(name="qx", bufs=1) as qxp,
            tc.tile_pool(name="qw", bufs=1) as qwp,
            tc.tile_pool(name="cx", bufs=4) as cxp,
            tc.tile_pool(name="xs", bufs=12) as xsp,
            tc.tile_pool(name="ws", bufs=2) as wsp,
            tc.tile_pool(name="scr", bufs=6) as scrp,
            tc.tile_pool(name="outp", bufs=6) as outp,
            tc.tile_pool(name="psum", bufs=6, space="PSUM") as psp,
        ):
            # ---- scale rows ----
            rnx_sb = cpool.tile([1, b], F32)
            rnw_sb = cpool.tile([1, m_sh], F32)
            sx_sb = cpool.tile([1, 1], F32)
            sw_sb = cpool.tile([1, 1], F32)
            nc.sync.dma_start(rnx_sb[:], rnx[:])
            nc.sync.dma_start(rnw_sb[:], rnw[:])
            nc.sync.dma_start(sx_sb[:], sx[:])
            nc.sync.dma_start(sw_sb[:], sw[:])

            # c = (rnorm / s) * 127   (quantization multiplier per row);
            # tensor_scalar has no divide op, so use reciprocal + mult
            rsx = cpool.tile([1, 1], F32)
            nc.vector.reciprocal(rsx[:], sx_sb[:])
            rsw = cpool.tile([1, 1], F32)
            nc.vector.reciprocal(rsw[:], sw_sb[:])
            nc.vector.tensor_scalar(
                rnx_sb[:], rnx_sb[:],
                scalar1=rsx[0:1, 0:1], scalar2=127.0,
                op0=ALU.mult, op1=ALU.mult,
            )
            nc.vector.tensor_scalar(
                rnw_sb[:], rnw_sb[:],
                scalar1=rsw[0:1, 0:1], scalar2=127.0,
                op0=ALU.mult, op1=ALU.mult,
            )
            # bounce via DRAM so the rows can be partition-broadcast by DMA
            cx_dram = dram.tile([1, b], F32)
            cw_dram = dram.tile([1, m_sh], F32)
            nc.sync.dma_start(cx_dram[:], rnx_sb[:])
            nc.sync.dma_start(cw_dram[:], rnw_sb[:])

            # epilogue scale: -(sx/127)*(sw/127), broadcast to all partitions
            nsxsw = cpool.tile([1, 1], F32)
            nc.vector.tensor_scalar(
                nsxsw[:], sx_sb[:],
                scalar1=sw_sb[0:1, 0:1], scalar2=-1.0 / (127.0 * 127.0),
                op0=ALU.mult, op1=ALU.mult,
            )
            nsxsw_b = cpool.tile([P, 1], F32)
            nc.gpsimd.partition_broadcast(nsxsw_b[:], nsxsw[:])

            # bias constants for the round-to-nearest magic trick
            kpos = cpool.tile([P, 1], F32)
            nc.vector.memset(kpos[:], KMAG)
            kneg = cpool.tile([P, 1], F32)
            nc.vector.memset(kneg[:], -KMAG)

            # ---- PE warmup: junk matmuls so the HAM clock gate is already
            # at full rate when the real stream starts (deps: only the memset)
            warm = cpool.tile([P, 512], BF16)
            nc.vector.memset(warm[:], 1.0)
            wps = psp.tile([P, n_free], F32, tag="warmps", name="warmps")
            for _ in range(20):
                nc.tensor.matmul(
                    wps[:], warm[:, 0:P], warm[:, 0:n_free], start=True, stop=True
                )

            # ---- quantize weight shard: qwT[k] [P, m_sh] bf16 ----
            cw_full = cpool.tile([P, m_sh], F32)
            nc.sync.dma_start(cw_full[:], cw_dram[0:1, :].to_broadcast((P, m_sh)))
            qw_tiles = [None] * kb
            qx_tiles = {}

            def quant_w(k):
                wt = wsp.tile([P, m_sh], F32, tag="wt", name=f"wt{k}")
                nc.sync.dma_start(wt[:], wT[k * P : (k + 1) * P, :])
                tq = wsp.tile([P, m_sh], F32, tag="wtq", name=f"wtq{k}")
                nc.vector.tensor_mul(tq[:], wt[:], cw_full[:])
                uq = wsp.tile([P, m_sh], F32, tag="wuq", name=f"wuq{k}")
                nc.scalar.activation(uq[:], tq[:], AF.Identity, bias=kpos[:])
                qw_k = qwp.tile([P, m_sh], BF16, tag=f"qw{k}", name=f"qw{k}")
                nc.scalar.activation(qw_k[:], uq[:], AF.Identity, bias=kneg[:])
                qw_tiles[k] = qw_k

            def quant_x(k, ch, cxf):
                xt = xsp.tile([P, b_chunk], F32, tag="xt", name=f"xt{k}_{ch}")
                nc.sync.dma_start(
                    xt[:], xT[k * P : (k + 1) * P, ch * b_chunk : (ch + 1) * b_chunk]
                )
                tq = scrp.tile([P, b_chunk], F32, tag="xtq", name=f"xtq{k}_{ch}")
                nc.vector.tensor_mul(tq[:], xt[:], cxf[:])
                uq = scrp.tile([P, b_chunk], F32, tag="xuq", name=f"xuq{k}_{ch}")
                nc.scalar.activation(uq[:], tq[:], AF.Identity, bias=kpos[:])
                qx_k = qxp.tile([P, b_chunk], BF16, tag=f"qx{k}_{ch}", name=f"qx{k}_{ch}")
                nc.scalar.activation(qx_k[:], uq[:], AF.Identity, bias=kneg[:])
                qx_tiles[(k, ch)] = qx_k

            def cxf_for(ch):
                cxf = cxp.tile([P, b_chunk], F32, tag="cxf", name=f"cxf{ch}")
                nc.sync.dma_start(
                    cxf[:],
                    cx_dram[0:1, ch * b_chunk : (ch + 1) * b_chunk].to_broadcast(
                        (P, b_chunk)
                    ),
                )
                return cxf

            def quant_chunk(ch):
                cxf = cxf_for(ch)
                for k in range(kb):
                    quant_x(k, ch, cxf)

            # startup: interleave w and x chunk-0 blocks so the first matmuls
            # (needing qw[k] and qx[k][0] in k order) unblock as early as possible
            cxf0 = cxf_for(0)
            for k in range(kb):
                quant_w(k)
                quant_x(k, 0, cxf0)
            for ch in (1, 2):
                if ch < nch:
                    quant_chunk(ch)
            for ch in range(nch):
                for bt in range(nbt_per_ch):
                    pss = [
                        psp.tile([P, n_free], F32, tag="mm", name=f"mm_{ch}_{bt}_{i}")
                        for i in range(nmt)
                    ]
                    lo = bt * P
                    for k in range(kb):
                        lhsT = qx_tiles[(k, ch)][:, lo : lo + P]
                        for mt in range(nmt):
                            nc.tensor.matmul(
                                pss[mt][:],
                                lhsT,
                                qw_tiles[k][:, mt * n_free : (mt + 1) * n_free],
                                start=(k == 0),
                                stop=(k == kb - 1),
                            )
                    row = ch * b_chunk + bt * P
                    for mt in range(nmt):
                        ot = outp.tile([P, n_free], F32, tag="ot")
                        nc.vector.tensor_scalar(
                            ot[:], pss[mt][:],
                            scalar1=nsxsw_b[:], scalar2=1.0,
                            op0=ALU.mult, op1=ALU.add,
                        )
                        nc.sync.dma_start(
                            out[row : row + P, mt * n_free : (mt + 1) * n_free], ot[:]
                        )
                if ch + 3 < nch:
                    quant_chunk(ch + 3)
    nc.compile()
    return nc


# --------------------------------------------------------------------------
# host orchestration
# --------------------------------------------------------------------------
def _pm_to_vec(a):
    """[P, nt] partition-major stats tile -> flat row vector (b = t*P + p)."""
    return np.ascontiguousarray(a.T).reshape(-1)


def kernel(x, weight):
    x = _f32(x)
    w = _f32(weight)
    b, d = x.shape
    m, d2 = w.shape
    assert (b, d, m, d2) == (B_FULL, D_FULL, M_FULL, D_FULL), (x.shape, w.shape)
    b_sl = b // N_CORES
    m_sh = m // N_CORES
    cores = list(range(N_CORES))

    # ---- launch 1: stats ----
    nc1 = _cached_program("stats", lambda: build_stats_program(b_sl, m_sh, d))
    in1 = [
        {
            "x_sl": np.ascontiguousarray(x[c * b_sl : (c + 1) * b_sl]),
            "w_sh": np.ascontiguousarray(w[c * m_sh : (c + 1) * m_sh]),
        }
        for c in cores
    ]
    res1 = run_bass_kernel_spmd(nc1, in1, core_ids=cores).results

    rn_x = np.concatenate([_pm_to_vec(res1[c]["x_rn"]) for c in cores])
    s_x = np.float32(max(np.float32(res1[c]["x_rm"][0, 0]) for c in cores))
    s_w = np.float32(max(np.float32(res1[c]["w_rm"][0, 0]) for c in cores))

    # ---- launch 2: quantize + matmul ----
    nc2 = _cached_program("main", lambda: build_main_program(b, m_sh, d))
    xT = np.ascontiguousarray(x.T)
    rnx_row = rn_x.reshape(1, b)
    sx_t = np.full((1, 1), s_x, dtype=np.float32)
    sw_t = np.full((1, 1), s_w, dtype=np.float32)
    in2 = []
    for c in cores:
        in2.append(
            {
                "xT": xT,
                "wT": np.ascontiguousarray(w[c * m_sh : (c + 1) * m_sh].T),
                "rnx": rnx_row,
                "rnw": _pm_to_vec(res1[c]["w_rn"]).reshape(1, m_sh),
                "sx": sx_t,
                "sw": sw_t,
            }
        )
    try:
        r = run_bass_kernel_spmd(nc2, in2, core_ids=cores, trace=TRACE)
    except ModuleNotFoundError:
        # axon NTFF profiling hook unavailable in this container
        r = run_bass_kernel_spmd(nc2, in2, core_ids=cores, trace=False)
    LAST["exec_time_ns"] = r.exec_time_ns
    LAST["mean_exec_time_ns"] = r.mean_exec_time_ns
    LAST["trace"] = r.instructions_and_trace[1] if r.instructions_and_trace else None
    LAST["in2"] = in2
    LAST["nc2"] = nc2

    return np.concatenate([r.results[c]["out"] for c in cores], axis=1)


# revision 23
# speedup vs baseline: 252.7517x; 22.3457x over previous
"""Trainium2 Bass kernel for quantized cosine-distance (1 - cos similarity).

Math: the reference bit-slices 7-bit symmetric-quantized, L2-normalized inputs
into (1,2,4)-bit groups and recombines 9 low-bit GEMMs with power-of-two
weights.  That recombination is exactly  qx @ qw^T  with qx, qw integer
matrices in [-127, 127].  Those integers are exact in bf16 and every partial
dot product over D=1024 is < 2^24, so a single bf16 matmul with f32 PSUM
accumulation reproduces the 9-GEMM result exactly.

Kernel structure (8 NeuronCores, weight sharded along M, x replicated):
  Launch 1 (tiny): per-core row stats (1/norm, max|row|/norm) for its x slice
      and weight shard.  Host only gathers shards and takes max of 8 scalars.
  Launch 2 (main): quantize x and w-shard in transposed layout, one big bf16
      GEMM per core -> [B, M/8] block, epilogue 1 - s*acc, DMA out.
"""

import os

import numpy as np

import concourse.bass as bass
import concourse.mybir as mybir
import concourse.tile as tile
from concourse import bacc
from concourse.bass_isa import ReduceOp
from concourse.bass_utils import run_bass_kernel_spmd

F32 = mybir.dt.float32
BF16 = mybir.dt.bfloat16
AF = mybir.ActivationFunctionType
ALU = mybir.AluOpType
AX = mybir.AxisListType

N_CORES = 8
B_FULL = 4096
D_FULL = 1024
M_FULL = 8192
P = 128

# magic constant: adding then subtracting 1.5*2^23 rounds |v|<2^22 to the
# nearest integer (ties-to-even), matching jnp.round for our value range
KMAG = float(np.float32(1.5 * 2**23))

# set by test.py to capture a profile of the main launch (NTFF hook is not
# available in all containers; falls back to no trace)
TRACE = bool(int(os.environ.get("COSDIST_TRACE", "0")))
LAST = {}
_PROGRAM_CACHE = {}


def _cached_program(key, builder):
    if key not in _PROGRAM_CACHE:
        _PROGRAM_CACHE[key] = builder()
    return _PROGRAM_CACHE[key]


def _f32(a):
    return np.ascontiguousarray(np.asarray(a, dtype=np.float32))


# --------------------------------------------------------------------------
# Launch 1: row stats.  Inputs per core: x_sl [B_SL, D], w_sh [M_SH, D].
# Outputs: rnorm (1/max(||row||,1e-12)) in [P, ntiles] partition-major layout
# and the per-core max of (max|row| / ||row||) as [1, 1].
# --------------------------------------------------------------------------
def build_stats_program(b_sl, m_sh, d):
    nc = bacc.Bacc("TRN2", target_bir_lowering=False, debug=False)
    x_sl = nc.dram_tensor("x_sl", [b_sl, d], F32, kind="ExternalInput")
    w_sh = nc.dram_tensor("w_sh", [m_sh, d], F32, kind="ExternalInput")
    x_rn = nc.dram_tensor("x_rn", [P, b_sl // P], F32, kind="ExternalOutput")
    x_rm = nc.dram_tensor("x_rm", [1, 1], F32, kind="ExternalOutput")
    w_rn = nc.dram_tensor("w_rn", [P, m_sh // P], F32, kind="ExternalOutput")
    w_rm = nc.dram_tensor("w_rm", [1, 1], F32, kind="ExternalOutput")

    with tile.TileContext(nc) as tc:
        with (
            tc.tile_pool(name="work", bufs=3) as work,
            tc.tile_pool(name="stat", bufs=1) as stat,
        ):
            for inp, nt, rn_out, rm_out, pre in (
                (x_sl, b_sl // P, x_rn, x_rm, "x"),
                (w_sh, m_sh // P, w_rn, w_rm, "w"),
            ):
                ssq = stat.tile([P, nt], F32, tag=f"{pre}ssq")
                amax = stat.tile([P, nt], F32, tag=f"{pre}amax")
                for t in range(nt):
                    xt = work.tile([P, d], F32, tag="xt")
                    nc.sync.dma_start(xt[:], inp[t * P : (t + 1) * P, :])
                    sq = work.tile([P, d], F32, tag="sq")
                    nc.vector.tensor_mul(sq[:], xt[:], xt[:])
                    nc.vector.tensor_reduce(
                        ssq[:, t : t + 1], sq[:], axis=AX.X, op=ALU.add
                    )
                    nc.vector.tensor_reduce(
                        amax[:, t : t + 1],
                        xt[:],
                        axis=AX.X,
                        op=ALU.max,
                        apply_absolute_value=True,
                    )
                norm = stat.tile([P, nt], F32, tag=f"{pre}norm")
                nc.scalar.sqrt(norm[:], ssq[:])
                nc.vector.tensor_scalar_max(norm[:], norm[:], 1e-12)
                rnorm = stat.tile([P, nt], F32, tag=f"{pre}rn")
                nc.vector.reciprocal(rnorm[:], norm[:])
                ratio = stat.tile([P, nt], F32, tag=f"{pre}ratio")
                nc.vector.tensor_mul(ratio[:], amax[:], rnorm[:])
                rmax = stat.tile([P, 1], F32, tag=f"{pre}rmax")
                nc.vector.tensor_reduce(rmax[:], ratio[:], axis=AX.X, op=ALU.max)
                gmax = stat.tile([P, 1], F32, tag=f"{pre}gmax")
                nc.gpsimd.partition_all_reduce(gmax[:], rmax[:], P, ReduceOp.max)
                nc.sync.dma_start(rn_out[:], rnorm[:])
                nc.sync.dma_start(rm_out[:], gmax[0:1, 0:1])
    nc.compile()
    return nc


# --------------------------------------------------------------------------
# Launch 2: quantize + GEMM + epilogue.
# Inputs per core (all transposed layouts prepared host-side):
#   xT   [D, B]     x transposed (replicated)
#   wT   [D, M_SH]  weight shard transposed
#   rnx  [1, B]     1/norm per x row (full)
#   rnw  [1, M_SH]  1/norm per weight row (this shard)
#   sx   [1, 1]     global max|xn|;  sw [1, 1] likewise for w
# Output: out [B, M_SH] = 1 - (sx/127)*(sw/127) * (qx @ qw^T) block
# --------------------------------------------------------------------------
def build_main_program(
    b, m_sh, d, n_free=512, b_chunk=512, repeats=1,
    epi_split=False,  # alternate epilogue between DVE and ACT (worse in model)
    w_k_on_dve=True,  # w-quant +K on DVE (relieves ACT startup backlog)
    mm_bufs=7,
):
    # repeats>1 re-emits the whole compute body N times in one NEFF, reusing
    # the same SBUF tiles (so passes serialize); used only to measure pure
    # execution time by differencing wall clock across repeat counts.
    nc = bacc.Bacc("TRN2", target_bir_lowering=False, debug=False)
    xT = nc.dram_tensor("xT", [d, b], F32, kind="ExternalInput")
    wT = nc.dram_tensor("wT", [d, m_sh], F32, kind="ExternalInput")
    rnx = nc.dram_tensor("rnx", [1, b], F32, kind="ExternalInput")
    rnw = nc.dram_tensor("rnw", [1, m_sh], F32, kind="ExternalInput")
    sx = nc.dram_tensor("sx", [1, 1], F32, kind="ExternalInput")
    sw = nc.dram_tensor("sw", [1, 1], F32, kind="ExternalInput")
    out = nc.dram_tensor("out", [b, m_sh], F32, kind="ExternalOutput")

    kb = d // P  # number of 128-deep contraction blocks
    nch = b // b_chunk  # b-chunks for pipelined x quantization
    nbt_per_ch = b_chunk // P  # 128-row output tiles per chunk
    nmt = m_sh // n_free  # output column tiles

    with tile.TileContext(nc) as tc:
        with (
            tc.tile_pool(name="dram", bufs=1, space="DRAM") as dram,
            tc.tile_pool(name="const", bufs=1) as cpool,
            tc.tile_pool(name="qx", bufs=1) as qxp,
            tc.tile_pool(name="qw", bufs=1) as qwp,
            tc.tile_pool(name="cx", bufs=4) as cxp,
            tc.tile_pool(name="xs", bufs=12) as xsp,
            tc.tile_pool(name="ws", bufs=2) as wsp,
            tc.tile_pool(name="scr", bufs=6) as scrp,
            tc.tile_pool(name="outp", bufs=6) as outp,
            tc.tile_pool(name="psum", bufs=mm_bufs, space="PSUM") as psp,
        ):
            # ---- scale rows ----
            rnx_sb = cpool.tile([1, b], F32)
            rnw_sb = cpool.tile([1, m_sh], F32)
            sx_sb = cpool.tile([1, 1], F32)
            sw_sb = cpool.tile([1, 1], F32)
            nc.sync.dma_start(rnx_sb[:], rnx[:])
            nc.sync.dma_start(rnw_sb[:], rnw[:])
            nc.sync.dma_start(sx_sb[:], sx[:])
            nc.sync.dma_start(sw_sb[:], sw[:])

            # c = (rnorm / s) * 127   (quantization multiplier per row);
            # tensor_scalar has no divide op, so use reciprocal + mult
            rsx = cpool.tile([1, 1], F32)
            nc.vector.reciprocal(rsx[:], sx_sb[:])
            rsw = cpool.tile([1, 1], F32)
            nc.vector.reciprocal(rsw[:], sw_sb[:])
            nc.vector.tensor_scalar(
                rnx_sb[:], rnx_sb[:],
                scalar1=rsx[0:1, 0:1], scalar2=127.0,
                op0=ALU.mult, op1=ALU.mult,
            )
            nc.vector.tensor_scalar(
                rnw_sb[:], rnw_sb[:],
                scalar1=rsw[0:1, 0:1], scalar2=127.0,
                op0=ALU.mult, op1=ALU.mult,
            )
            # bounce via DRAM so the rows can be partition-broadcast by DMA
            cx_dram = dram.tile([1, b], F32)
            cw_dram = dram.tile([1, m_sh], F32)
            nc.sync.dma_start(cx_dram[:], rnx_sb[:])
            nc.sync.dma_start(cw_dram[:], rnw_sb[:])

            # epilogue scale: -(sx/127)*(sw/127), broadcast to all partitions
            nsxsw = cpool.tile([1, 1], F32)
            nc.vector.tensor_scalar(
                nsxsw[:], sx_sb[:],
                scalar1=sw_sb[0:1, 0:1], scalar2=-1.0 / (127.0 * 127.0),
                op0=ALU.mult, op1=ALU.mult,
            )
            nsxsw_b = cpool.tile([P, 1], F32)
            nc.gpsimd.partition_broadcast(nsxsw_b[:], nsxsw[:])

            # bias constants for the round-to-nearest magic trick
            kpos = cpool.tile([P, 1], F32)
            nc.vector.memset(kpos[:], KMAG)
            kneg = cpool.tile([P, 1], F32)
            nc.vector.memset(kneg[:], -KMAG)
            ones_b = cpool.tile([P, 1], F32)
            nc.vector.memset(ones_b[:], 1.0)

            # ---- PE warmup: junk matmuls so the HAM clock gate is already
            # at full rate when the real stream starts (deps: only the memset)
            warm = cpool.tile([P, 512], BF16)
            nc.vector.memset(warm[:], 1.0)
            wps = psp.tile([P, n_free], F32, tag="warmps", name="warmps", bufs=1)
            for _ in range(20):
                nc.tensor.matmul(
                    wps[:], warm[:, 0:P], warm[:, 0:n_free], start=True, stop=True
                )

            # ---- quantize weight shard: qwT[k] [P, m_sh] bf16 ----
            cw_full = cpool.tile([P, m_sh], F32)
            nc.sync.dma_start(cw_full[:], cw_dram[0:1, :].to_broadcast((P, m_sh)))

            def body(rep):
                qw_tiles = [None] * kb
                qx_tiles = {}

                def quant_w(k):
                    wt = wsp.tile([P, m_sh], F32, tag="wt", name=f"wt{k}r{rep}")
                    nc.sync.dma_start(wt[:], wT[k * P : (k + 1) * P, :])
                    tq = wsp.tile([P, m_sh], F32, tag="wtq", name=f"wtq{k}r{rep}")
                    nc.vector.tensor_mul(tq[:], wt[:], cw_full[:])
                    uq = wsp.tile([P, m_sh], F32, tag="wuq", name=f"wuq{k}r{rep}")
                    if w_k_on_dve:
                        nc.vector.tensor_scalar_add(uq[:], tq[:], KMAG)
                    else:
                        nc.scalar.activation(uq[:], tq[:], AF.Identity, bias=kpos[:])
                    qw_k = qwp.tile([P, m_sh], BF16, tag=f"qw{k}", name=f"qw{k}r{rep}")
                    nc.scalar.activation(qw_k[:], uq[:], AF.Identity, bias=kneg[:])
                    qw_tiles[k] = qw_k

                def quant_x(k, ch, cxf):
                    xt = xsp.tile([P, b_chunk], F32, tag="xt", name=f"xt{k}_{ch}r{rep}")
                    nc.sync.dma_start(
                        xt[:],
                        xT[k * P : (k + 1) * P, ch * b_chunk : (ch + 1) * b_chunk],
                    )
                    tq = scrp.tile(
                        [P, b_chunk], F32, tag="xtq", name=f"xtq{k}_{ch}r{rep}"
                    )
                    nc.vector.tensor_mul(tq[:], xt[:], cxf[:])
                    uq = scrp.tile(
                        [P, b_chunk], F32, tag="xuq", name=f"xuq{k}_{ch}r{rep}"
                    )
                    nc.scalar.activation(uq[:], tq[:], AF.Identity, bias=kpos[:])
                    qx_k = qxp.tile(
                        [P, b_chunk], BF16, tag=f"qx{k}_{ch}", name=f"qx{k}_{ch}r{rep}"
                    )
                    nc.scalar.activation(qx_k[:], uq[:], AF.Identity, bias=kneg[:])
                    qx_tiles[(k, ch)] = qx_k

                def cxf_for(ch):
                    cxf = cxp.tile([P, b_chunk], F32, tag="cxf", name=f"cxf{ch}r{rep}")
                    nc.sync.dma_start(
                        cxf[:],
                        cx_dram[0:1, ch * b_chunk : (ch + 1) * b_chunk].to_broadcast(
                            (P, b_chunk)
                        ),
                    )
                    return cxf

                def quant_chunk(ch):
                    cxf = cxf_for(ch)
                    for k in range(kb):
                        quant_x(k, ch, cxf)

                # startup: interleave w and x chunk-0 blocks so the first
                # matmuls (needing qw[k] and qx[k][0] in k order) unblock early
                cxf0 = cxf_for(0)
                for k in range(kb):
                    quant_w(k)
                    quant_x(k, 0, cxf0)
                for ch in (1, 2):
                    if ch < nch:
                        quant_chunk(ch)
                for ch in range(nch):
                    for bt in range(nbt_per_ch):
                        pss = [
                            psp.tile(
                                [P, n_free],
                                F32,
                                tag="mm",
                                name=f"mm_{ch}_{bt}_{i}r{rep}",
                            )
                            for i in range(nmt)
                        ]
                        lo = bt * P
                        for k in range(kb):
                            lhsT = qx_tiles[(k, ch)][:, lo : lo + P]
                            for mt in range(nmt):
                                nc.tensor.matmul(
                                    pss[mt][:],
                                    lhsT,
                                    qw_tiles[k][:, mt * n_free : (mt + 1) * n_free],
                                    start=(k == 0),
                                    stop=(k == kb - 1),
                                )
                        row = ch * b_chunk + bt * P
                        for mt in range(nmt):
                            ot = outp.tile(
                                [P, n_free], F32, tag="ot", name=f"ot_{ch}_{bt}_{mt}r{rep}"
                            )
                            # epilogue: out = 1 + acc * (-sx*sw), alternating
                            # between DVE and ACT so PSUM banks drain via two
                            # independent engines
                            if epi_split and (bt + mt) % 2 == 0:
                                nc.scalar.activation(
                                    ot[:], pss[mt][:], AF.Identity,
                                    bias=ones_b[:], scale=nsxsw_b[:],
                                )
                            else:
                                nc.vector.tensor_scalar(
                                    ot[:], pss[mt][:],
                                    scalar1=nsxsw_b[:], scalar2=1.0,
                                    op0=ALU.mult, op1=ALU.add,
                                )
                            nc.sync.dma_start(
                                out[row : row + P, mt * n_free : (mt + 1) * n_free],
                                ot[:],
                            )
                    if ch + 3 < nch:
                        quant_chunk(ch + 3)

            for rep in range(repeats):
                body(rep)
    nc.compile()
    return nc


# --------------------------------------------------------------------------
# host orchestration
# --------------------------------------------------------------------------
def _pm_to_vec(a):
    """[P, nt] partition-major stats tile -> flat row vector (b = t*P + p)."""
    return np.ascontiguousarray(a.T).reshape(-1)


def kernel(x, weight):
    x = _f32(x)
    w = _f32(weight)
    b, d = x.shape
    m, d2 = w.shape
    assert (b, d, m, d2) == (B_FULL, D_FULL, M_FULL, D_FULL), (x.shape, w.shape)
    b_sl = b // N_CORES
    m_sh = m // N_CORES
    cores = list(range(N_CORES))

    # ---- launch 1: stats ----
    nc1 = _cached_program("stats", lambda: build_stats_program(b_sl, m_sh, d))
    in1 = [
        {
            "x_sl": np.ascontiguousarray(x[c * b_sl : (c + 1) * b_sl]),
            "w_sh": np.ascontiguousarray(w[c * m_sh : (c + 1) * m_sh]),
        }
        for c in cores
    ]
    res1 = run_bass_kernel_spmd(nc1, in1, core_ids=cores).results

    rn_x = np.concatenate([_pm_to_vec(res1[c]["x_rn"]) for c in cores])
    s_x = np.float32(max(np.float32(res1[c]["x_rm"][0, 0]) for c in cores))
    s_w = np.float32(max(np.float32(res1[c]["w_rm"][0, 0]) for c in cores))

    # ---- launch 2: quantize + matmul ----
    nc2 = _cached_program("main", lambda: build_main_program(b, m_sh, d))
    xT = np.ascontiguousarray(x.T)
    rnx_row = rn_x.reshape(1, b)
    sx_t = np.full((1, 1), s_x, dtype=np.float32)
    sw_t = np.full((1, 1), s_w, dtype=np.float32)
    in2 = []
    for c in cores:
        in2.append(
            {
                "xT": xT,
                "wT": np.ascontiguousarray(w[c * m_sh : (c + 1) * m_sh].T),
                "rnx": rnx_row,
                "rnw": _pm_to_vec(res1[c]["w_rn"]).reshape(1, m_sh),
                "sx": sx_t,
                "sw": sw_t,
            }
        )
    try:
        r = run_bass_kernel_spmd(nc2, in2, core_ids=cores, trace=TRACE)
    except ModuleNotFoundError:
        # axon NTFF profiling hook unavailable in this container
        r = run_bass_kernel_spmd(nc2, in2, core_ids=cores, trace=False)
    LAST["exec_time_ns"] = r.exec_time_ns
    LAST["mean_exec_time_ns"] = r.mean_exec_time_ns
    LAST["trace"] = r.instructions_and_trace[1] if r.instructions_and_trace else None
    LAST["in2"] = in2
    LAST["nc2"] = nc2

    return np.concatenate([r.results[c]["out"] for c in cores], axis=1)
